# revision 34
# baseline (speedup 1.0000x reference)
"""Causal flash attention for trn2: B=4,H=16,S=4096,D=64 fp32.

Sharding: 64 (b,h) heads -> 8 per NeuronCore, no cross-core comm.
Host prep (not counted in HW time): Q/K transposed to [d,s] bf16 (Q
pre-scaled by 1/sqrt(D)) and duplicated across both 64-row partition
halves; V pre-laid-out [128, nkt, D+1] bf16 with an appended
ones-column so the PV matmul also produces the softmax normalizer.

Per head, per 512-query block j (PSUM budget: 3 rotating 2-bank score
sets + 2 single-bank output accumulators = 8 banks), key tiles
t<=4j+3 in groups of 2:
  - ST[k=128, 1024] = two QK matmuls (contraction d=64, bf16, N=512).
    The two tiles of a group sit in opposite PE row-group halves
    (tile_position auto-derived from base partition 0 / 64), so they
    execute concurrently in the array on hardware.
  - causal masking is folded into the PE stream: an accumulating
    identity @ mask matmul adds -60 to future-key entries of the
    diagonal 128x128 block, so exp yields ~0 there; fully-masked
    columns are skipped entirely (QK + exp width restriction).
  - exp in ONE instruction per group, engines alternating 3:2:
      ACT: exp activation (table)                        -> pt bf16
      DVE: Schraudolph bit-trick exp (x*A+B -> int16 = bf16 bits),
           max rel err ~3%, cancels in the softmax ratio; measured
           final rel err ~1.1e-2 vs 2e-2 budget
  - PV per 128-query sub s: O[q=128, s*65..+65] += PT_s^T @ [V_t|1].
    One PSUM bank holds all 4 sub-accumulators: the single start=True
    clears has_written once, later first-touches overwrite.
  - software pipeline: PV lags 4 groups behind QK/exp and a block's
    trailing PV groups + epilogue drain interleaved between the NEXT
    block's (or next head's) QK groups, so the in-order PE stream
    always holds independent matmul work while exp results land.
Epilogue per block: one strided reciprocal of the 4 normalizer
columns, 4 scaled copies, one gather-DMA [128, 4, 64] -> out rows,
issued from the idle GPSIMD queue so stores never block the SP input
prefetch queue.

TimelineSim: 401.4 us/core; engine busy ~ PE 356 (88%) / ACT 352 (87%)
/ DVE 325 (81%) us. HW-measured (reps-loop slope, see test.py):
~455-520 us/pass depending on co-tenant load -- the kernel is exp-
throughput-bound (ACT ~129 G elem/s from PSUM + DVE ~93 G elem/s, 75.5M
score elems/core => ~350 us floor). PE p-state ramp is chewed by dummy
warmup matmuls during cold-start. DMA issues serialize ~650ns each on
the DGE config engine, so cold-start loads are chunked and ordered by
first use. _build(reps=R, unroll=U) wraps the pass in a tc.For_i
hardware loop (U passes unrolled per iteration to amortize the ~16us
back-edge all-engine barrier) -- profiling-only; the graded kernel()
path uses reps=1 (no loop).
TimelineSim trace analysis (duck-typed recorder): sim says PE busy
354us / ACT 352us, but the sim cost model ignores tile_position and
serializes the dual-half QK pairs. HW microbench (pe_probe-style):
dual-half pairs really overlap -- 291 ns/pair vs 884 serial (3x), and
forcing the full kernel serial costs 848 vs 458 us/pass. So on HW the
PE has slack and ACT (~352us busy) is THE critical engine.
KEY FIX (-47us/pass, 467->420 interleaved A/B): diagonal groups used
to emit QK_a, mask_a, QK_b, mask_b -- the full-128-row mask matmul
between the two 64-row QK halves broke their overlap window at every
diagonal group. MASK_DEFER emits both QK halves adjacent, then the
masks. Same mechanism explains TILE_DRAIN's +19us (PV matmuls between
the halves).
Rejected with data: fp16 PSUM scores (bass requires fp32 matmul out),
GT=1/GT=3 grouping (GT=3 re-rejected on HW: 588 vs 491 us/pass),
greedy exp assignment, shared big score tile (serializes on the dep
tracker), within-group exp splits, tri-mask on DVE/Pool (stream
serialization / no PSUM port). HW A/B re-checks (interleaved, same
run): exp 1:1 and 2:1 ACT:DVE splits, PV lag 4/6/7, pt bufs 12 -- all
tied or worse than the 3:2 / lag 5 / bufs 8 baseline; re-swept after
the MASK_DEFER fix (4:3 +36us, lag 6 +28us -- 3:2/lag5 still wins).
BLOCK_DESC +3.3us in sim. fp8 QK DoubleRow pointless: PE has slack
once the dual-half overlap works, and e4m3 score error (~4%) risks
the 2e-2 budget. Epilogue can't leave DVE: o_ps is PSUM and
Pool/GpSimd has no PSUM port; ACT is the longer pole.
HW-measured after MASK_DEFER: 385-420 us/pass (phase-dependent),
vs ~352us ACT-busy floor.
DIAG_REVERSE (-5.3 us/pass, interleaved A/B): higher-dg tile first in
diagonal groups so the exp e0 cut skips more masked cols and the
second tile's span is fully valid (256 fewer exp'd cols per block,
zero extra instructions). Requires emission-order-aware PV stop flags
(last writer per o_ps region). GOTCHA: PSUM start=True clears
has_written for the WHOLE bank -- exactly one start per o_ps bank
(the block's first PV matmul); a start per region wipes earlier
regions' accumulation (0.9 rel err). The last block of the last head
drains its PV eagerly (lag 1) -- no future QK work exists for the lag
to protect, shrinking the serialized tail.
Engine calibration on HW (reps-slope): all-ACT 562us = solo-rate
prediction at 130.6 G elem/s (zero in-kernel ACT stall); all-DVE
681us = 108 G elem/s. Mixed ACT+DVE runs at only ~180 G/s combined
(75% of additive) and the penalty grows toward balanced splits -- the
3:2 five-cycle is a sharp optimum (ratio swept 50..100% ACT; 57% +36,
58.3% +6, 62.5% +8, 67% +50 us vs 3:2). The ~408-414us span is this
mixed-engine contention floor, not a schedule defect. A third exp
consumer (GPSIMD via PSUM->SBUF DMA bounce) is impossible: dma_start
asserts source is SBUF/DRAM -- DMA has no PSUM read path, so scores
can only leave PSUM through ACT or DVE, and a copy costs either
engine the same as the exp itself.
"""

import math
from contextlib import ExitStack

import numpy as np
import ml_dtypes

B, H, S, D = 4, 16, 4096, 64
NCORES = 8
HPC = (B * H) // NCORES  # heads per core
QB = 512                 # query block
KT = 128                 # key tile (PE partition dim)
NKT = S // KT            # 32 key tiles per head
GT = 2                   # key tiles per exp group
DV = D + 1               # value cols + normalizer ones-column

# Schraudolph exp constants for bf16 bit pattern (7-bit mantissa):
# bits = round(x * 128/ln2 + (127*128 - 366393/65536))
SCH_A = 128.0 / math.log(2.0)
SCH_B = 127 * 128 - 366393.0 / 65536.0

# exp engine schedule: cycle over groups; ~3:2 ACT:DVE
EXP_PATTERN = ("dve", "act", "act", "dve", "act")
PV_LAG = 5  # groups the PV matmuls trail behind QK/exp within a block
ST_BUFS = 3  # rotating PSUM score sets (GT*QB*4B each; 8 banks total)
BLOCK_DESC = False  # h>0 heads process query blocks j descending
TILE_DRAIN = False  # drain carried PV per key-tile (finer PE interleave)
DUAL_HALF = True    # QK pair tiles in opposite PE row-group halves
MASK_DEFER = True   # emit diagonal mask matmuls after both QK halves
DIAG_REVERSE = True  # higher-dg tile first in diagonal groups (less exp)
DIAG_ENG = None      # force last-2 groups' exp engines, e.g. ('act','dve')

_cache = {}


def _build(causal: bool, hpc: int = HPC, s_len: int = S, reps: int = 1,
           unroll: int = 1):
    import concourse.tile as tile
    from concourse import bacc, mybir

    f32 = mybir.dt.float32
    f16 = mybir.dt.float16
    bf16 = mybir.dt.bfloat16
    i16 = mybir.dt.int16
    EXP = mybir.ActivationFunctionType.Exp
    MULT = mybir.AluOpType.mult
    ADD = mybir.AluOpType.add
    nkt_total = s_len // KT
    nqb = s_len // QB

    nc = bacc.Bacc("TRN2", target_bir_lowering=False)
    qt_d = nc.dram_tensor("qt", [hpc, 2 * D, s_len], bf16, kind="ExternalInput")
    kt_d = nc.dram_tensor("kt", [hpc, 2 * D, s_len], bf16, kind="ExternalInput")
    v_d = nc.dram_tensor("v", [hpc, KT, nkt_total, DV], bf16, kind="ExternalInput")
    tri_d = nc.dram_tensor("tri", [KT, 2 * KT], bf16, kind="ExternalInput")
    o_d = nc.dram_tensor("o", [hpc, s_len, D], f32, kind="ExternalOutput")

    with ExitStack() as ctx:
        tc = ctx.enter_context(tile.TileContext(nc))
        qk_pool = ctx.enter_context(tc.tile_pool(name="qk", bufs=2))
        v_pool = ctx.enter_context(tc.tile_pool(name="v", bufs=2))
        p_pool = ctx.enter_context(tc.tile_pool(name="p", bufs=8))
        st_pool = ctx.enter_context(
            tc.tile_pool(name="st", bufs=ST_BUFS, space="PSUM"))
        o_pool = ctx.enter_context(tc.tile_pool(name="oacc", bufs=2, space="PSUM"))
        out_pool = ctx.enter_context(tc.tile_pool(name="out", bufs=4))
        const_pool = ctx.enter_context(tc.tile_pool(name="const", bufs=1))

        tri_t = const_pool.tile([KT, 2 * KT], bf16)

        # dummy matmuls chew the PE p-state ramp during the cold-start DMA
        # wait, so the first real QK matmuls run at full clock (reads
        # uninitialized SBUF; output lands in an st bank that the first
        # start=True QK clears before anything reads it)
        warm_st = st_pool.tile([KT, GT * QB], f32, tag="st", name="warm_st")
        for _ in range(10):
            nc.tensor.matmul(
                warm_st[:, :KT], tri_t[:, :KT], tri_t[:, :KT],
                start=True, stop=True,
            )

        if reps > 1:
            # profiling-only: repeat the whole attention pass in a hardware
            # loop so wall-clock slope over reps isolates true HW exec time
            # from the per-launch dispatch overhead (~1ms through axon).
            # The body is unrolled `unroll` passes per loop iteration to
            # amortize the For_i back-edge all-engine barrier (~16us).
            assert reps % unroll == 0
            ctx.enter_context(tc.For_i(0, reps // unroll, 1))

        for _rep in range(unroll if reps > 1 else 1):
            _emit_pass(nc, causal, hpc, s_len, qt_d, kt_d, v_d, o_d, tri_d,
                       tri_t, qk_pool, v_pool, p_pool, st_pool, o_pool,
                       out_pool, f32, bf16, i16, EXP, MULT, ADD,
                       nkt_total, nqb, _rep)
    nc.compile()
    return nc


def _emit_pass(nc, causal, hpc, s_len, qt_d, kt_d, v_d, o_d, tri_d, tri_t,
               qk_pool, v_pool, p_pool, st_pool, o_pool, out_pool,
               f32, bf16, i16, EXP, MULT, ADD, nkt_total, nqb, rep):
        gcounter = 0
        carry = []        # PV-pending groups of the previous block
        carry_epi = None  # (h, j, o_ps) of the previous block
        for h in range(hpc):
            qt_t = qk_pool.tile([2 * D, s_len], bf16, tag="qt")
            kt_t = qk_pool.tile([2 * D, s_len], bf16, tag="kt")
            if h == 0 and rep == 0:
                # split the cold-start loads so block j=0 can begin while the
                # rest streams in; DMA *issues* serialize at ~650ns each on
                # the DGE config engine, so order by first use (tri is only
                # needed by the j0/t0 mask matmul, after the first QK)
                c = 2 * QB
                nc.sync.dma_start(out=qt_t[:, :c], in_=qt_d[h, :, :c])
                nc.sync.dma_start(out=kt_t[:, :c], in_=kt_d[h, :, :c])
                nc.sync.dma_start(out=tri_t, in_=tri_d[:])
                # v precedes the bulk Q/K loads: the first PV (lag 4) needs
                # V around ~4us in, well before blocks j>=2 need the far
                # K columns
                v_t = v_pool.tile([KT, nkt_total, DV], bf16, tag="v")
                nc.sync.dma_start(out=v_t, in_=v_d[h])
                nc.sync.dma_start(out=qt_t[:, c:], in_=qt_d[h, :, c:])
                nc.sync.dma_start(out=kt_t[:, c:], in_=kt_d[h, :, c:])
            else:
                nc.sync.dma_start(out=qt_t, in_=qt_d[h])
                nc.sync.dma_start(out=kt_t, in_=kt_d[h])
                v_t = v_pool.tile([KT, nkt_total, DV], bf16, tag="v")
                nc.sync.dma_start(out=v_t, in_=v_d[h])

            # Cross-block software pipeline (carried across heads too): a
            # block's trailing PV groups and its epilogue are emitted
            # interleaved between the next block's QK/exp groups, so the
            # in-order PE stream always has independent QK work between PV
            # ops that wait on exp results.
            lag_units = PV_LAG * (GT if TILE_DRAIN else 1)
            blocks = list(range(nqb - 1, -1, -1) if (BLOCK_DESC and h > 0)
                          else range(nqb))
            for j in blocks:
                # last block of the last head has no future QK work for the
                # PV lag to protect: drain eagerly to shrink the tail
                lag_eff = (1 if (h == hpc - 1 and j == blocks[-1])
                           else lag_units)
                o_ps = o_pool.tile([KT, 4 * DV], f32, tag="oacc",
                                   name=f"o_{rep}_{h}_{j}")
                nkt = 4 * (j + 1) if causal else nkt_total
                groups = [
                    list(range(g0, min(g0 + GT, nkt))) for g0 in range(0, nkt, GT)
                ]
                if causal and DIAG_REVERSE:
                    # higher-dg tile first inside diagonal groups: the e0 cut
                    # then skips more leading masked cols and the second
                    # tile's span is fully valid -> 256 fewer exp'd cols per
                    # block at zero instruction cost
                    groups = [list(reversed(ts)) if ts[0] - 4 * j >= 0 else ts
                              for ts in groups]
                # PV accumulation flags: start=True ONLY on the block's
                # very first PV matmul (start clears has_written for the
                # whole PSUM bank; later region first-touches overwrite),
                # stop=True on each region's last writer in emission order
                # (reversal changes who that is)
                ts0 = groups[0][0]
                dg0_blk = ts0 - 4 * j if causal else -1
                first_pv = (ts0, max(dg0_blk, 0))
                last_w = {}
                for ts in groups:
                    for t in ts:
                        dg = t - 4 * j if causal else -1
                        for s in range(4):
                            if dg > s:
                                continue
                            last_w[s] = t
                pts = []

                def _drain_one():
                    nonlocal carry_epi
                    if carry:
                        _emit_pv(nc, causal, carry.pop(0))
                        if not carry and carry_epi is not None:
                            _emit_epilogue(nc, out_pool, o_d, carry_epi, f32)
                            carry_epi = None
                    elif len(pts) >= lag_eff:
                        _emit_pv(nc, causal, pts.pop(0))

                for ts in groups:
                    w = len(ts) * QB
                    # skip the leading fully-masked columns of the group's
                    # first tile (nothing reads them); interior stale spans
                    # of later tiles still get exp'd harmlessly
                    dg0 = ts[0] - 4 * j if causal else -1
                    e0 = max(dg0, 0) * KT
                    st = st_pool.tile([KT, GT * QB], f32, tag="st")
                    masks = []
                    for i, t in enumerate(ts):
                        dg = t - 4 * j if causal else -1
                        # diagonal tiles: only queries >= dg*128 can see keys
                        # of tile t; skip the fully-masked cols (stale PSUM
                        # there is finite, exp'd harmlessly, never read by PV)
                        q0 = max(dg, 0) * KT
                        # PE row-group half for concurrency (DUAL_HALF off
                        # forces both tiles into half 0 -> serial: HW probe).
                        # Measured on HW: the overlap is real and worth ~390
                        # us/pass (848 serial vs 458 dual).
                        r0 = (i % 2) * D if DUAL_HALF else 0
                        nc.tensor.matmul(
                            st[:, i * QB + q0:(i + 1) * QB],
                            kt_t[r0:r0 + D, t * KT:(t + 1) * KT],
                            qt_t[r0:r0 + D, j * QB + q0:(j + 1) * QB],
                            start=True, stop=(dg < 0),
                        )
                        if dg >= 0:
                            if MASK_DEFER:
                                masks.append((i, dg))
                            else:
                                c0 = i * QB + dg * KT
                                nc.tensor.matmul(
                                    st[:, c0:c0 + KT],
                                    tri_t[:, :KT],
                                    tri_t[:, KT:2 * KT],
                                    start=False, stop=True,
                                )
                        if TILE_DRAIN:
                            # one PV tile after each QK tile: halves the PE
                            # burst quantum at the drain points
                            _drain_one()
                    for i, dg in masks:
                        # add -60 to masked (future-key) entries of the
                        # diagonal 128x128 block: identity.T @ mask_neg.
                        # Emitted AFTER both QK halves so the pair stays
                        # adjacent and the dual-half overlap window holds.
                        c0 = i * QB + dg * KT
                        nc.tensor.matmul(
                            st[:, c0:c0 + KT],
                            tri_t[:, :KT],
                            tri_t[:, KT:2 * KT],
                            start=False, stop=True,
                        )
                    if not TILE_DRAIN:
                        _drain_one()
                    pt = p_pool.tile([KT, GT * QB], bf16, tag="pt")
                    gi = groups.index(ts)
                    if DIAG_ENG is not None and causal and gi >= len(groups) - 2:
                        eng = DIAG_ENG[gi - (len(groups) - 2)]
                    else:
                        eng = EXP_PATTERN[gcounter % len(EXP_PATTERN)]
                        gcounter += 1
                    if eng == "dve":
                        nc.vector.tensor_scalar(
                            pt.bitcast(i16)[:, e0:w], st[:, e0:w], SCH_A, SCH_B,
                            MULT, ADD,
                        )
                    else:
                        nc.scalar.activation(pt[:, e0:w], st[:, e0:w], EXP)
                    if TILE_DRAIN:
                        for i, t in enumerate(ts):
                            pts.append((j, o_ps, [t], pt, v_t, i,
                                        first_pv, last_w))
                    else:
                        pts.append((j, o_ps, ts, pt, v_t, 0,
                                    first_pv, last_w))
                # previous block fully drained by now (it has fewer
                # groups than this block); stash this block's backlog
                for grp in carry:
                    _emit_pv(nc, causal, grp)
                if carry_epi is not None:
                    _emit_epilogue(nc, out_pool, o_d, carry_epi, f32)
                carry = pts
                carry_epi = (h, j, o_ps, rep)
        for grp in carry:
            _emit_pv(nc, causal, grp)
        if carry_epi is not None:
            _emit_epilogue(nc, out_pool, o_d, carry_epi, f32)


def _emit_pv(nc, causal, group):
    j, o_ps, ts, pt, v_t, i0, first_pv, last_w = group
    for i, t in enumerate(ts):
        ii = i0 + i  # column base of tile t within the group's pt tile
        dg = t - 4 * j if causal else -1
        for s in range(4):
            if dg > s:
                continue
            nc.tensor.matmul(
                o_ps[:, s * DV:(s + 1) * DV],
                pt[:, ii * QB + s * KT:ii * QB + (s + 1) * KT],
                v_t[:, t, :],
                start=((t, s) == first_pv),
                stop=(t == last_w[s]),
            )


def _emit_epilogue(nc, out_pool, o_d, epi, f32):
    h, j, o_ps, rep = epi
    recip = out_pool.tile([KT, 4], f32, tag="recip", name=f"r_{rep}_{h}_{j}")
    nc.vector.reciprocal(recip, o_ps[:, D::DV])
    out_t = out_pool.tile([KT, 4, D], f32, tag="out", name=f"t_{rep}_{h}_{j}")
    for s in range(4):
        nc.vector.tensor_scalar_mul(
            out_t[:, s], o_ps[:, s * DV:s * DV + D], recip[:, s:s + 1]
        )
    # issued from the idle GPSIMD queue so stores never block input-prefetch
    # DMAs queued on SP
    nc.gpsimd.dma_start(
        out=o_d[h, j * QB:(j + 1) * QB, :].rearrange("(s p) d -> p s d", s=4),
        in_=out_t,
    )


last_results = None  # BassKernelResults of the most recent run (for test.py)


def _make_in_maps(query, key, value):
    bf = ml_dtypes.bfloat16
    # cast to bf16 BEFORE the transposes: halves the bytes shuffled by the
    # non-contiguous copies below (host prep wall time)
    q4 = (np.asarray(query, dtype=np.float32) / math.sqrt(D)).astype(bf) \
        .reshape(B * H, S, D)
    k4 = np.asarray(key, dtype=np.float32).astype(bf).reshape(B * H, S, D)
    v4 = np.asarray(value, dtype=np.float32).astype(bf).reshape(B * H, S, D)
    # [identity | strict-lower-tri * -60]: operands of the mask-add matmul
    # (identity.T @ mask adds -60 where query < key inside a diagonal block)
    tri = np.concatenate(
        [np.eye(KT, dtype=np.float32),
         np.tril(np.full((KT, KT), -60.0, dtype=np.float32), -1)], axis=1
    ).astype(bf)

    in_maps = []
    for c in range(NCORES):
        sl = slice(c * HPC, (c + 1) * HPC)
        qt1 = np.ascontiguousarray(q4[sl].transpose(0, 2, 1))
        qt = np.concatenate([qt1, qt1], axis=1)
        kt1 = np.ascontiguousarray(k4[sl].transpose(0, 2, 1))
        kt = np.concatenate([kt1, kt1], axis=1)
        vb = v4[sl].reshape(HPC, NKT, KT, D)
        vones = np.empty((HPC, NKT, KT, DV), dtype=bf)
        vones[..., :D] = vb
        vones[..., D] = 1
        v_lay = np.ascontiguousarray(vones.transpose(0, 2, 1, 3))  # [HPC, KT, NKT, DV]
        in_maps.append({
            "qt": qt,
            "kt": kt,
            "v": v_lay,
            "tri": tri,
        })
    return in_maps


def _assemble(per_core_results):
    out = np.stack([r["o"] for r in per_core_results])  # [8, HPC, S, D]
    return np.ascontiguousarray(
        out.reshape(B, H, S, D)
    ).astype(np.float32)


def kernel(query, key, value, causal_mask):
    import os
    os.environ["BASS_NEVER_TRACE"] = "1"  # axon NTFF hook unavailable here
    from concourse.bass_utils import run_bass_kernel_spmd

    global last_results
    causal = bool(np.asarray(causal_mask).item())
    if causal not in _cache:
        _cache[causal] = _build(causal)
    nc = _cache[causal]

    in_maps = _make_in_maps(query, key, value)
    res = run_bass_kernel_spmd(nc, in_maps, core_ids=list(range(NCORES)))
    last_results = res
    return _assemble(res.results)



# revision 35
# speedup vs baseline: 1.0026x; 1.0026x over previous
"""Causal flash attention for trn2: B=4,H=16,S=4096,D=64 fp32.

Sharding: 64 (b,h) heads -> 8 per NeuronCore, no cross-core comm.
Host prep (not counted in HW time): Q/K transposed to [d,s] bf16 (Q
pre-scaled by 1/sqrt(D)) and duplicated across both 64-row partition
halves; V pre-laid-out [128, nkt, D+1] bf16 with an appended
ones-column so the PV matmul also produces the softmax normalizer.

Per head, per 512-query block j (PSUM budget: 3 rotating 2-bank score
sets + 2 single-bank output accumulators = 8 banks), key tiles
t<=4j+3 in groups of 2:
  - ST[k=128, 1024] = two QK matmuls (contraction d=64, bf16, N=512).
    The two tiles of a group sit in opposite PE row-group halves
    (tile_position auto-derived from base partition 0 / 64), so they
    execute concurrently in the array on hardware.
  - causal masking is folded into the PE stream: an accumulating
    identity @ mask matmul adds -60 to future-key entries of the
    diagonal 128x128 block, so exp yields ~0 there; fully-masked
    columns are skipped entirely (QK + exp width restriction).
  - exp in ONE instruction per group, engines alternating 3:2:
      ACT: exp activation (table)                        -> pt bf16
      DVE: Schraudolph bit-trick exp (x*A+B -> int16 = bf16 bits),
           max rel err ~3%, cancels in the softmax ratio; measured
           final rel err ~1.1e-2 vs 2e-2 budget
  - PV per 128-query sub s: O[q=128, s*65..+65] += PT_s^T @ [V_t|1].
    One PSUM bank holds all 4 sub-accumulators: the single start=True
    clears has_written once, later first-touches overwrite.
  - software pipeline: PV lags 4 groups behind QK/exp and a block's
    trailing PV groups + epilogue drain interleaved between the NEXT
    block's (or next head's) QK groups, so the in-order PE stream
    always holds independent matmul work while exp results land.
Epilogue per block: one strided reciprocal of the 4 normalizer
columns, 4 scaled copies, one gather-DMA [128, 4, 64] -> out rows,
issued from the idle GPSIMD queue so stores never block the SP input
prefetch queue.

TimelineSim: 401.4 us/core; engine busy ~ PE 356 (88%) / ACT 352 (87%)
/ DVE 325 (81%) us. HW-measured (reps-loop slope, see test.py):
~455-520 us/pass depending on co-tenant load -- the kernel is exp-
throughput-bound (ACT ~129 G elem/s from PSUM + DVE ~93 G elem/s, 75.5M
score elems/core => ~350 us floor). PE p-state ramp is chewed by dummy
warmup matmuls during cold-start. DMA issues serialize ~650ns each on
the DGE config engine, so cold-start loads are chunked and ordered by
first use. _build(reps=R, unroll=U) wraps the pass in a tc.For_i
hardware loop (U passes unrolled per iteration to amortize the ~16us
back-edge all-engine barrier) -- profiling-only; the graded kernel()
path uses reps=1 (no loop).
TimelineSim trace analysis (duck-typed recorder): sim says PE busy
354us / ACT 352us, but the sim cost model ignores tile_position and
serializes the dual-half QK pairs. HW microbench (pe_probe-style):
dual-half pairs really overlap -- 291 ns/pair vs 884 serial (3x), and
forcing the full kernel serial costs 848 vs 458 us/pass. So on HW the
PE has slack and ACT (~352us busy) is THE critical engine.
KEY FIX (-47us/pass, 467->420 interleaved A/B): diagonal groups used
to emit QK_a, mask_a, QK_b, mask_b -- the full-128-row mask matmul
between the two 64-row QK halves broke their overlap window at every
diagonal group. MASK_DEFER emits both QK halves adjacent, then the
masks. Same mechanism explains TILE_DRAIN's +19us (PV matmuls between
the halves).
Rejected with data: fp16 PSUM scores (bass requires fp32 matmul out),
GT=1/GT=3 grouping (GT=3 re-rejected on HW: 588 vs 491 us/pass),
greedy exp assignment, shared big score tile (serializes on the dep
tracker), within-group exp splits, tri-mask on DVE/Pool (stream
serialization / no PSUM port). HW A/B re-checks (interleaved, same
run): exp 1:1 and 2:1 ACT:DVE splits, PV lag 4/6/7, pt bufs 12 -- all
tied or worse than the 3:2 / lag 5 / bufs 8 baseline; re-swept after
the MASK_DEFER fix (4:3 +36us, lag 6 +28us -- 3:2/lag5 still wins).
BLOCK_DESC +3.3us in sim. fp8 QK DoubleRow pointless: PE has slack
once the dual-half overlap works, and e4m3 score error (~4%) risks
the 2e-2 budget. Epilogue can't leave DVE: o_ps is PSUM and
Pool/GpSimd has no PSUM port; ACT is the longer pole.
HW-measured after MASK_DEFER: 385-420 us/pass (phase-dependent),
vs ~352us ACT-busy floor.
DIAG_REVERSE (-5.3 us/pass, interleaved A/B): higher-dg tile first in
diagonal groups so the exp e0 cut skips more masked cols and the
second tile's span is fully valid (256 fewer exp'd cols per block,
zero extra instructions). Requires emission-order-aware PV stop flags
(last writer per o_ps region). GOTCHA: PSUM start=True clears
has_written for the WHOLE bank -- exactly one start per o_ps bank
(the block's first PV matmul); a start per region wipes earlier
regions' accumulation (0.9 rel err). The last block of the last head
drains its PV eagerly (lag 1) -- no future QK work exists for the lag
to protect, shrinking the serialized tail.
Engine calibration on HW (reps-slope): all-ACT 562us = solo-rate
prediction at 130.6 G elem/s (zero in-kernel ACT stall); all-DVE
681us = 108 G elem/s. Mixed ACT+DVE runs at only ~180 G/s combined
(75% of additive) and the penalty grows toward balanced splits -- the
3:2 five-cycle is a sharp optimum (ratio swept 50..100% ACT; 57% +36,
58.3% +6, 62.5% +8, 67% +50 us vs 3:2). The ~408-414us span is this
mixed-engine contention floor, not a schedule defect. A third exp
consumer (GPSIMD via PSUM->SBUF DMA bounce) is impossible: dma_start
asserts source is SBUF/DRAM -- DMA has no PSUM read path, so scores
can only leave PSUM through ACT or DVE, and a copy costs either
engine the same as the exp itself.
"""

import math
from contextlib import ExitStack

import numpy as np
import ml_dtypes

B, H, S, D = 4, 16, 4096, 64
NCORES = 8
HPC = (B * H) // NCORES  # heads per core
QB = 512                 # query block
KT = 128                 # key tile (PE partition dim)
NKT = S // KT            # 32 key tiles per head
GT = 2                   # key tiles per exp group
DV = D + 1               # value cols + normalizer ones-column

# Schraudolph exp constants for bf16 bit pattern (7-bit mantissa):
# bits = round(x * 128/ln2 + (127*128 - 366393/65536))
SCH_A = 128.0 / math.log(2.0)
SCH_B = 127 * 128 - 366393.0 / 65536.0

# exp engine schedule: cycle over groups; ~3:2 ACT:DVE
EXP_PATTERN = ("dve", "act", "act", "dve", "act")
PV_LAG = 5  # groups the PV matmuls trail behind QK/exp within a block
ST_BUFS = 3  # rotating PSUM score sets (GT*QB*4B each; 8 banks total)
BLOCK_DESC = False  # h>0 heads process query blocks j descending
TILE_DRAIN = False  # drain carried PV per key-tile (finer PE interleave)
DUAL_HALF = True    # QK pair tiles in opposite PE row-group halves
MASK_DEFER = True   # emit diagonal mask matmuls after both QK halves
DIAG_REVERSE = True  # higher-dg tile first in diagonal groups (less exp)
DIAG_ENG = None      # force last-2 groups' exp engines, e.g. ('act','dve')

_cache = {}


def _build(causal: bool, hpc: int = HPC, s_len: int = S, reps: int = 1,
           unroll: int = 1):
    import concourse.tile as tile
    from concourse import bacc, mybir

    f32 = mybir.dt.float32
    f16 = mybir.dt.float16
    bf16 = mybir.dt.bfloat16
    i16 = mybir.dt.int16
    EXP = mybir.ActivationFunctionType.Exp
    MULT = mybir.AluOpType.mult
    ADD = mybir.AluOpType.add
    nkt_total = s_len // KT
    nqb = s_len // QB

    nc = bacc.Bacc("TRN2", target_bir_lowering=False)
    qt_d = nc.dram_tensor("qt", [hpc, 2 * D, s_len], bf16, kind="ExternalInput")
    kt_d = nc.dram_tensor("kt", [hpc, 2 * D, s_len], bf16, kind="ExternalInput")
    v_d = nc.dram_tensor("v", [hpc, KT, nkt_total, DV], bf16, kind="ExternalInput")
    tri_d = nc.dram_tensor("tri", [KT, 2 * KT], bf16, kind="ExternalInput")
    o_d = nc.dram_tensor("o", [hpc, s_len, D], f32, kind="ExternalOutput")

    with ExitStack() as ctx:
        tc = ctx.enter_context(tile.TileContext(nc))
        qk_pool = ctx.enter_context(tc.tile_pool(name="qk", bufs=3))
        v_pool = ctx.enter_context(tc.tile_pool(name="v", bufs=2))
        p_pool = ctx.enter_context(tc.tile_pool(name="p", bufs=8))
        st_pool = ctx.enter_context(
            tc.tile_pool(name="st", bufs=ST_BUFS, space="PSUM"))
        o_pool = ctx.enter_context(tc.tile_pool(name="oacc", bufs=2, space="PSUM"))
        out_pool = ctx.enter_context(tc.tile_pool(name="out", bufs=4))
        const_pool = ctx.enter_context(tc.tile_pool(name="const", bufs=1))

        tri_t = const_pool.tile([KT, 2 * KT], bf16)

        # dummy matmuls chew the PE p-state ramp during the cold-start DMA
        # wait, so the first real QK matmuls run at full clock (reads
        # uninitialized SBUF; output lands in an st bank that the first
        # start=True QK clears before anything reads it)
        warm_st = st_pool.tile([KT, GT * QB], f32, tag="st", name="warm_st")
        for _ in range(10):
            nc.tensor.matmul(
                warm_st[:, :KT], tri_t[:, :KT], tri_t[:, :KT],
                start=True, stop=True,
            )

        if reps > 1:
            # profiling-only: repeat the whole attention pass in a hardware
            # loop so wall-clock slope over reps isolates true HW exec time
            # from the per-launch dispatch overhead (~1ms through axon).
            # The body is unrolled `unroll` passes per loop iteration to
            # amortize the For_i back-edge all-engine barrier (~16us).
            assert reps % unroll == 0
            ctx.enter_context(tc.For_i(0, reps // unroll, 1))

        for _rep in range(unroll if reps > 1 else 1):
            _emit_pass(nc, causal, hpc, s_len, qt_d, kt_d, v_d, o_d, tri_d,
                       tri_t, qk_pool, v_pool, p_pool, st_pool, o_pool,
                       out_pool, f32, bf16, i16, EXP, MULT, ADD,
                       nkt_total, nqb, _rep)
    nc.compile()
    return nc


def _emit_pass(nc, causal, hpc, s_len, qt_d, kt_d, v_d, o_d, tri_d, tri_t,
               qk_pool, v_pool, p_pool, st_pool, o_pool, out_pool,
               f32, bf16, i16, EXP, MULT, ADD, nkt_total, nqb, rep):
        gcounter = 0
        carry = []        # PV-pending groups of the previous block
        carry_epi = None  # (h, j, o_ps) of the previous block
        for h in range(hpc):
            qt_t = qk_pool.tile([2 * D, s_len], bf16, tag="qt")
            kt_t = qk_pool.tile([2 * D, s_len], bf16, tag="kt")
            if h == 0 and rep == 0:
                # split the cold-start loads so block j=0 can begin while the
                # rest streams in; DMA *issues* serialize at ~650ns each on
                # the DGE config engine, so order by first use (tri is only
                # needed by the j0/t0 mask matmul, after the first QK)
                c = 2 * QB
                nc.sync.dma_start(out=qt_t[:, :c], in_=qt_d[h, :, :c])
                nc.sync.dma_start(out=kt_t[:, :c], in_=kt_d[h, :, :c])
                nc.sync.dma_start(out=tri_t, in_=tri_d[:])
                # v precedes the bulk Q/K loads: the first PV (lag 4) needs
                # V around ~4us in, well before blocks j>=2 need the far
                # K columns
                v_t = v_pool.tile([KT, nkt_total, DV], bf16, tag="v")
                nc.sync.dma_start(out=v_t, in_=v_d[h])
                nc.sync.dma_start(out=qt_t[:, c:], in_=qt_d[h, :, c:])
                nc.sync.dma_start(out=kt_t[:, c:], in_=kt_d[h, :, c:])
            else:
                nc.sync.dma_start(out=qt_t, in_=qt_d[h])
                nc.sync.dma_start(out=kt_t, in_=kt_d[h])
                v_t = v_pool.tile([KT, nkt_total, DV], bf16, tag="v")
                nc.sync.dma_start(out=v_t, in_=v_d[h])

            # Cross-block software pipeline (carried across heads too): a
            # block's trailing PV groups and its epilogue are emitted
            # interleaved between the next block's QK/exp groups, so the
            # in-order PE stream always has independent QK work between PV
            # ops that wait on exp results.
            lag_units = PV_LAG * (GT if TILE_DRAIN else 1)
            blocks = list(range(nqb - 1, -1, -1) if (BLOCK_DESC and h > 0)
                          else range(nqb))
            for j in blocks:
                # last block of the last head has no future QK work for the
                # PV lag to protect: drain eagerly to shrink the tail
                lag_eff = (1 if (h == hpc - 1 and j == blocks[-1])
                           else lag_units)
                o_ps = o_pool.tile([KT, 4 * DV], f32, tag="oacc",
                                   name=f"o_{rep}_{h}_{j}")
                nkt = 4 * (j + 1) if causal else nkt_total
                groups = [
                    list(range(g0, min(g0 + GT, nkt))) for g0 in range(0, nkt, GT)
                ]
                if causal and DIAG_REVERSE:
                    # higher-dg tile first inside diagonal groups: the e0 cut
                    # then skips more leading masked cols and the second
                    # tile's span is fully valid -> 256 fewer exp'd cols per
                    # block at zero instruction cost
                    groups = [list(reversed(ts)) if ts[0] - 4 * j >= 0 else ts
                              for ts in groups]
                # PV accumulation flags: start=True ONLY on the block's
                # very first PV matmul (start clears has_written for the
                # whole PSUM bank; later region first-touches overwrite),
                # stop=True on each region's last writer in emission order
                # (reversal changes who that is)
                ts0 = groups[0][0]
                dg0_blk = ts0 - 4 * j if causal else -1
                first_pv = (ts0, max(dg0_blk, 0))
                last_w = {}
                for ts in groups:
                    for t in ts:
                        dg = t - 4 * j if causal else -1
                        for s in range(4):
                            if dg > s:
                                continue
                            last_w[s] = t
                pts = []

                def _drain_one():
                    nonlocal carry_epi
                    if carry:
                        _emit_pv(nc, causal, carry.pop(0))
                        if not carry and carry_epi is not None:
                            _emit_epilogue(nc, out_pool, o_d, carry_epi, f32)
                            carry_epi = None
                    elif len(pts) >= lag_eff:
                        _emit_pv(nc, causal, pts.pop(0))

                for ts in groups:
                    w = len(ts) * QB
                    # skip the leading fully-masked columns of the group's
                    # first tile (nothing reads them); interior stale spans
                    # of later tiles still get exp'd harmlessly
                    dg0 = ts[0] - 4 * j if causal else -1
                    e0 = max(dg0, 0) * KT
                    st = st_pool.tile([KT, GT * QB], f32, tag="st")
                    masks = []
                    for i, t in enumerate(ts):
                        dg = t - 4 * j if causal else -1
                        # diagonal tiles: only queries >= dg*128 can see keys
                        # of tile t; skip the fully-masked cols (stale PSUM
                        # there is finite, exp'd harmlessly, never read by PV)
                        q0 = max(dg, 0) * KT
                        # PE row-group half for concurrency (DUAL_HALF off
                        # forces both tiles into half 0 -> serial: HW probe).
                        # Measured on HW: the overlap is real and worth ~390
                        # us/pass (848 serial vs 458 dual).
                        r0 = (i % 2) * D if DUAL_HALF else 0
                        nc.tensor.matmul(
                            st[:, i * QB + q0:(i + 1) * QB],
                            kt_t[r0:r0 + D, t * KT:(t + 1) * KT],
                            qt_t[r0:r0 + D, j * QB + q0:(j + 1) * QB],
                            start=True, stop=(dg < 0),
                        )
                        if dg >= 0:
                            if MASK_DEFER:
                                masks.append((i, dg))
                            else:
                                c0 = i * QB + dg * KT
                                nc.tensor.matmul(
                                    st[:, c0:c0 + KT],
                                    tri_t[:, :KT],
                                    tri_t[:, KT:2 * KT],
                                    start=False, stop=True,
                                )
                        if TILE_DRAIN:
                            # one PV tile after each QK tile: halves the PE
                            # burst quantum at the drain points
                            _drain_one()
                    for i, dg in masks:
                        # add -60 to masked (future-key) entries of the
                        # diagonal 128x128 block: identity.T @ mask_neg.
                        # Emitted AFTER both QK halves so the pair stays
                        # adjacent and the dual-half overlap window holds.
                        c0 = i * QB + dg * KT
                        nc.tensor.matmul(
                            st[:, c0:c0 + KT],
                            tri_t[:, :KT],
                            tri_t[:, KT:2 * KT],
                            start=False, stop=True,
                        )
                    if not TILE_DRAIN:
                        _drain_one()
                    pt = p_pool.tile([KT, GT * QB], bf16, tag="pt")
                    gi = groups.index(ts)
                    if DIAG_ENG is not None and causal and gi >= len(groups) - 2:
                        eng = DIAG_ENG[gi - (len(groups) - 2)]
                    else:
                        eng = EXP_PATTERN[gcounter % len(EXP_PATTERN)]
                        gcounter += 1
                    if eng == "dve":
                        nc.vector.tensor_scalar(
                            pt.bitcast(i16)[:, e0:w], st[:, e0:w], SCH_A, SCH_B,
                            MULT, ADD,
                        )
                    else:
                        nc.scalar.activation(pt[:, e0:w], st[:, e0:w], EXP)
                    if TILE_DRAIN:
                        for i, t in enumerate(ts):
                            pts.append((j, o_ps, [t], pt, v_t, i,
                                        first_pv, last_w))
                    else:
                        pts.append((j, o_ps, ts, pt, v_t, 0,
                                    first_pv, last_w))
                # previous block fully drained by now (it has fewer
                # groups than this block); stash this block's backlog
                for grp in carry:
                    _emit_pv(nc, causal, grp)
                if carry_epi is not None:
                    _emit_epilogue(nc, out_pool, o_d, carry_epi, f32)
                carry = pts
                carry_epi = (h, j, o_ps, rep)
        for grp in carry:
            _emit_pv(nc, causal, grp)
        if carry_epi is not None:
            _emit_epilogue(nc, out_pool, o_d, carry_epi, f32)


def _emit_pv(nc, causal, group):
    j, o_ps, ts, pt, v_t, i0, first_pv, last_w = group
    for i, t in enumerate(ts):
        ii = i0 + i  # column base of tile t within the group's pt tile
        dg = t - 4 * j if causal else -1
        for s in range(4):
            if dg > s:
                continue
            nc.tensor.matmul(
                o_ps[:, s * DV:(s + 1) * DV],
                pt[:, ii * QB + s * KT:ii * QB + (s + 1) * KT],
                v_t[:, t, :],
                start=((t, s) == first_pv),
                stop=(t == last_w[s]),
            )


def _emit_epilogue(nc, out_pool, o_d, epi, f32):
    h, j, o_ps, rep = epi
    recip = out_pool.tile([KT, 4], f32, tag="recip", name=f"r_{rep}_{h}_{j}")
    nc.vector.reciprocal(recip, o_ps[:, D::DV])
    out_t = out_pool.tile([KT, 4, D], f32, tag="out", name=f"t_{rep}_{h}_{j}")
    for s in range(4):
        nc.vector.tensor_scalar_mul(
            out_t[:, s], o_ps[:, s * DV:s * DV + D], recip[:, s:s + 1]
        )
    # issued from the idle GPSIMD queue so stores never block input-prefetch
    # DMAs queued on SP
    nc.gpsimd.dma_start(
        out=o_d[h, j * QB:(j + 1) * QB, :].rearrange("(s p) d -> p s d", s=4),
        in_=out_t,
    )


last_results = None  # BassKernelResults of the most recent run (for test.py)


def _make_in_maps(query, key, value):
    bf = ml_dtypes.bfloat16
    # cast to bf16 BEFORE the transposes: halves the bytes shuffled by the
    # non-contiguous copies below (host prep wall time)
    q4 = (np.asarray(query, dtype=np.float32) / math.sqrt(D)).astype(bf) \
        .reshape(B * H, S, D)
    k4 = np.asarray(key, dtype=np.float32).astype(bf).reshape(B * H, S, D)
    v4 = np.asarray(value, dtype=np.float32).astype(bf).reshape(B * H, S, D)
    # [identity | strict-lower-tri * -60]: operands of the mask-add matmul
    # (identity.T @ mask adds -60 where query < key inside a diagonal block)
    tri = np.concatenate(
        [np.eye(KT, dtype=np.float32),
         np.tril(np.full((KT, KT), -60.0, dtype=np.float32), -1)], axis=1
    ).astype(bf)

    in_maps = []
    for c in range(NCORES):
        sl = slice(c * HPC, (c + 1) * HPC)
        qt1 = np.ascontiguousarray(q4[sl].transpose(0, 2, 1))
        qt = np.concatenate([qt1, qt1], axis=1)
        kt1 = np.ascontiguousarray(k4[sl].transpose(0, 2, 1))
        kt = np.concatenate([kt1, kt1], axis=1)
        vb = v4[sl].reshape(HPC, NKT, KT, D)
        vones = np.empty((HPC, NKT, KT, DV), dtype=bf)
        vones[..., :D] = vb
        vones[..., D] = 1
        v_lay = np.ascontiguousarray(vones.transpose(0, 2, 1, 3))  # [HPC, KT, NKT, DV]
        in_maps.append({
            "qt": qt,
            "kt": kt,
            "v": v_lay,
            "tri": tri,
        })
    return in_maps


def _assemble(per_core_results):
    out = np.stack([r["o"] for r in per_core_results])  # [8, HPC, S, D]
    return np.ascontiguousarray(
        out.reshape(B, H, S, D)
    ).astype(np.float32)


def kernel(query, key, value, causal_mask):
    import os
    os.environ["BASS_NEVER_TRACE"] = "1"  # axon NTFF hook unavailable here
    from concourse.bass_utils import run_bass_kernel_spmd

    global last_results
    causal = bool(np.asarray(causal_mask).item())
    if causal not in _cache:
        _cache[causal] = _build(causal)
    nc = _cache[causal]

    in_maps = _make_in_maps(query, key, value)
    res = run_bass_kernel_spmd(nc, in_maps, core_ids=list(range(NCORES)))
    last_results = res
    return _assemble(res.results)



# revision 36
# speedup vs baseline: 1.0160x; 1.0133x over previous
"""Causal flash attention for trn2: B=4,H=16,S=4096,D=64 fp32.

Sharding: 64 (b,h) heads -> 8 per NeuronCore, no cross-core comm.
Host prep (not counted in HW time): Q/K transposed to [d,s] bf16 (Q
pre-scaled by 1/sqrt(D)) and duplicated across both 64-row partition
halves; V pre-laid-out [128, nkt, D+1] bf16 with an appended
ones-column so the PV matmul also produces the softmax normalizer.

Per head, per 512-query block j (PSUM budget: 3 rotating 2-bank score
sets + 2 single-bank output accumulators = 8 banks), key tiles
t<=4j+3 in groups of 2:
  - ST[k=128, 1024] = two QK matmuls (contraction d=64, bf16, N=512).
    The two tiles of a group sit in opposite PE row-group halves
    (tile_position auto-derived from base partition 0 / 64), so they
    execute concurrently in the array on hardware.
  - causal masking is folded into the PE stream: an accumulating
    identity @ mask matmul adds -60 to future-key entries of the
    diagonal 128x128 block, so exp yields ~0 there; fully-masked
    columns are skipped entirely (QK + exp width restriction).
  - exp in ONE instruction per group, engines alternating 3:2:
      ACT: exp activation (table)                        -> pt bf16
      DVE: Schraudolph bit-trick exp (x*A+B -> int16 = bf16 bits),
           max rel err ~3%, cancels in the softmax ratio; measured
           final rel err ~1.1e-2 vs 2e-2 budget
  - PV per 128-query sub s: O[q=128, s*65..+65] += PT_s^T @ [V_t|1].
    One PSUM bank holds all 4 sub-accumulators: the single start=True
    clears has_written once, later first-touches overwrite.
  - software pipeline: PV lags 4 groups behind QK/exp and a block's
    trailing PV groups + epilogue drain interleaved between the NEXT
    block's (or next head's) QK groups, so the in-order PE stream
    always holds independent matmul work while exp results land.
Epilogue per block: one strided reciprocal of the 4 normalizer
columns, 4 scaled copies, one gather-DMA [128, 4, 64] -> out rows,
issued from the idle GPSIMD queue so stores never block the SP input
prefetch queue.

TimelineSim: 401.4 us/core; engine busy ~ PE 356 (88%) / ACT 352 (87%)
/ DVE 325 (81%) us. HW-measured (reps-loop slope, see test.py):
~455-520 us/pass depending on co-tenant load -- the kernel is exp-
throughput-bound (ACT ~129 G elem/s from PSUM + DVE ~93 G elem/s, 75.5M
score elems/core => ~350 us floor). PE p-state ramp is chewed by dummy
warmup matmuls during cold-start. DMA issues serialize ~650ns each on
the DGE config engine, so cold-start loads are chunked and ordered by
first use. _build(reps=R, unroll=U) wraps the pass in a tc.For_i
hardware loop (U passes unrolled per iteration to amortize the ~16us
back-edge all-engine barrier) -- profiling-only; the graded kernel()
path uses reps=1 (no loop).
TimelineSim trace analysis (duck-typed recorder): sim says PE busy
354us / ACT 352us, but the sim cost model ignores tile_position and
serializes the dual-half QK pairs. HW microbench (pe_probe-style):
dual-half pairs really overlap -- 291 ns/pair vs 884 serial (3x), and
forcing the full kernel serial costs 848 vs 458 us/pass. So on HW the
PE has slack and ACT (~352us busy) is THE critical engine.
KEY FIX (-47us/pass, 467->420 interleaved A/B): diagonal groups used
to emit QK_a, mask_a, QK_b, mask_b -- the full-128-row mask matmul
between the two 64-row QK halves broke their overlap window at every
diagonal group. MASK_DEFER emits both QK halves adjacent, then the
masks. Same mechanism explains TILE_DRAIN's +19us (PV matmuls between
the halves).
Rejected with data: fp16 PSUM scores (bass requires fp32 matmul out),
GT=1/GT=3 grouping (GT=3 re-rejected on HW: 588 vs 491 us/pass),
greedy exp assignment, shared big score tile (serializes on the dep
tracker), within-group exp splits, tri-mask on DVE/Pool (stream
serialization / no PSUM port). HW A/B re-checks (interleaved, same
run): exp 1:1 and 2:1 ACT:DVE splits, PV lag 4/6/7, pt bufs 12 -- all
tied or worse than the 3:2 / lag 5 / bufs 8 baseline; re-swept after
the MASK_DEFER fix (4:3 +36us, lag 6 +28us -- 3:2/lag5 still wins).
BLOCK_DESC +3.3us in sim. fp8 QK DoubleRow pointless: PE has slack
once the dual-half overlap works, and e4m3 score error (~4%) risks
the 2e-2 budget. Epilogue can't leave DVE: o_ps is PSUM and
Pool/GpSimd has no PSUM port; ACT is the longer pole.
HW-measured after MASK_DEFER: 385-420 us/pass (phase-dependent),
vs ~352us ACT-busy floor.
DIAG_REVERSE (-5.3 us/pass, interleaved A/B): higher-dg tile first in
diagonal groups so the exp e0 cut skips more masked cols and the
second tile's span is fully valid (256 fewer exp'd cols per block,
zero extra instructions). Requires emission-order-aware PV stop flags
(last writer per o_ps region). GOTCHA: PSUM start=True clears
has_written for the WHOLE bank -- exactly one start per o_ps bank
(the block's first PV matmul); a start per region wipes earlier
regions' accumulation (0.9 rel err). The last block of the last head
drains its PV eagerly (lag 1) -- no future QK work exists for the lag
to protect, shrinking the serialized tail.
Engine calibration on HW (reps-slope): all-ACT 562us = solo-rate
prediction at 130.6 G elem/s (zero in-kernel ACT stall); all-DVE
681us = 108 G elem/s. Mixed ACT+DVE runs at only ~180 G/s combined
(75% of additive) and the penalty grows toward balanced splits -- the
3:2 five-cycle is a sharp optimum (ratio swept 50..100% ACT; 57% +36,
58.3% +6, 62.5% +8, 67% +50 us vs 3:2). The ~408-414us span is this
mixed-engine contention floor, not a schedule defect. A third exp
consumer (GPSIMD via PSUM->SBUF DMA bounce) is impossible: dma_start
asserts source is SBUF/DRAM -- DMA has no PSUM read path, so scores
can only leave PSUM through ACT or DVE, and a copy costs either
engine the same as the exp itself.
Forced diagonal exp-engine assignment (DIAG_ENG (act,dve)/(dve,act))
+4.2/+6.2 us vs the rotating pattern -- phase diversity wins.
qk_pool bufs=3 (-2.5 us, A/B): input prefetch runs two heads ahead,
smoothing the per-pass DMA schedule.
"""

import math
from contextlib import ExitStack

import numpy as np
import ml_dtypes

B, H, S, D = 4, 16, 4096, 64
NCORES = 8
HPC = (B * H) // NCORES  # heads per core
QB = 512                 # query block
KT = 128                 # key tile (PE partition dim)
NKT = S // KT            # 32 key tiles per head
GT = 2                   # key tiles per exp group
DV = D + 1               # value cols + normalizer ones-column

# Schraudolph exp constants for bf16 bit pattern (7-bit mantissa):
# bits = round(x * 128/ln2 + (127*128 - 366393/65536))
SCH_A = 128.0 / math.log(2.0)
SCH_B = 127 * 128 - 366393.0 / 65536.0

# exp engine schedule: cycle over groups; ~3:2 ACT:DVE
EXP_PATTERN = ("dve", "act", "act", "dve", "act")
PV_LAG = 5  # groups the PV matmuls trail behind QK/exp within a block
ST_BUFS = 3  # rotating PSUM score sets (GT*QB*4B each; 8 banks total)
BLOCK_DESC = False  # h>0 heads process query blocks j descending
TILE_DRAIN = False  # drain carried PV per key-tile (finer PE interleave)
DUAL_HALF = True    # QK pair tiles in opposite PE row-group halves
MASK_DEFER = True   # emit diagonal mask matmuls after both QK halves
DIAG_REVERSE = True  # higher-dg tile first in diagonal groups (less exp)
DIAG_ENG = None      # force last-2 groups' exp engines, e.g. ('act','dve')

_cache = {}


def _build(causal: bool, hpc: int = HPC, s_len: int = S, reps: int = 1,
           unroll: int = 1):
    import concourse.tile as tile
    from concourse import bacc, mybir

    f32 = mybir.dt.float32
    f16 = mybir.dt.float16
    bf16 = mybir.dt.bfloat16
    i16 = mybir.dt.int16
    EXP = mybir.ActivationFunctionType.Exp
    MULT = mybir.AluOpType.mult
    ADD = mybir.AluOpType.add
    nkt_total = s_len // KT
    nqb = s_len // QB

    nc = bacc.Bacc("TRN2", target_bir_lowering=False)
    qt_d = nc.dram_tensor("qt", [hpc, 2 * D, s_len], bf16, kind="ExternalInput")
    kt_d = nc.dram_tensor("kt", [hpc, 2 * D, s_len], bf16, kind="ExternalInput")
    v_d = nc.dram_tensor("v", [hpc, KT, nkt_total, DV], bf16, kind="ExternalInput")
    tri_d = nc.dram_tensor("tri", [KT, 2 * KT], bf16, kind="ExternalInput")
    o_d = nc.dram_tensor("o", [hpc, s_len, D], f32, kind="ExternalOutput")

    with ExitStack() as ctx:
        tc = ctx.enter_context(tile.TileContext(nc))
        qk_pool = ctx.enter_context(tc.tile_pool(name="qk", bufs=3))
        v_pool = ctx.enter_context(tc.tile_pool(name="v", bufs=2))
        p_pool = ctx.enter_context(tc.tile_pool(name="p", bufs=8))
        st_pool = ctx.enter_context(
            tc.tile_pool(name="st", bufs=ST_BUFS, space="PSUM"))
        o_pool = ctx.enter_context(tc.tile_pool(name="oacc", bufs=2, space="PSUM"))
        out_pool = ctx.enter_context(tc.tile_pool(name="out", bufs=4))
        const_pool = ctx.enter_context(tc.tile_pool(name="const", bufs=1))

        tri_t = const_pool.tile([KT, 2 * KT], bf16)

        # dummy matmuls chew the PE p-state ramp during the cold-start DMA
        # wait, so the first real QK matmuls run at full clock (reads
        # uninitialized SBUF; output lands in an st bank that the first
        # start=True QK clears before anything reads it)
        warm_st = st_pool.tile([KT, GT * QB], f32, tag="st", name="warm_st")
        for _ in range(10):
            nc.tensor.matmul(
                warm_st[:, :KT], tri_t[:, :KT], tri_t[:, :KT],
                start=True, stop=True,
            )

        if reps > 1:
            # profiling-only: repeat the whole attention pass in a hardware
            # loop so wall-clock slope over reps isolates true HW exec time
            # from the per-launch dispatch overhead (~1ms through axon).
            # The body is unrolled `unroll` passes per loop iteration to
            # amortize the For_i back-edge all-engine barrier (~16us).
            assert reps % unroll == 0
            ctx.enter_context(tc.For_i(0, reps // unroll, 1))

        for _rep in range(unroll if reps > 1 else 1):
            _emit_pass(nc, causal, hpc, s_len, qt_d, kt_d, v_d, o_d, tri_d,
                       tri_t, qk_pool, v_pool, p_pool, st_pool, o_pool,
                       out_pool, f32, bf16, i16, EXP, MULT, ADD,
                       nkt_total, nqb, _rep)
    nc.compile()
    return nc


def _emit_pass(nc, causal, hpc, s_len, qt_d, kt_d, v_d, o_d, tri_d, tri_t,
               qk_pool, v_pool, p_pool, st_pool, o_pool, out_pool,
               f32, bf16, i16, EXP, MULT, ADD, nkt_total, nqb, rep):
        gcounter = 0
        carry = []        # PV-pending groups of the previous block
        carry_epi = None  # (h, j, o_ps) of the previous block
        for h in range(hpc):
            qt_t = qk_pool.tile([2 * D, s_len], bf16, tag="qt")
            kt_t = qk_pool.tile([2 * D, s_len], bf16, tag="kt")
            if h == 0 and rep == 0:
                # split the cold-start loads so block j=0 can begin while the
                # rest streams in; DMA *issues* serialize at ~650ns each on
                # the DGE config engine, so order by first use (tri is only
                # needed by the j0/t0 mask matmul, after the first QK)
                c = 2 * QB
                nc.sync.dma_start(out=qt_t[:, :c], in_=qt_d[h, :, :c])
                nc.sync.dma_start(out=kt_t[:, :c], in_=kt_d[h, :, :c])
                nc.sync.dma_start(out=tri_t, in_=tri_d[:])
                # v precedes the bulk Q/K loads: the first PV (lag 4) needs
                # V around ~4us in, well before blocks j>=2 need the far
                # K columns
                v_t = v_pool.tile([KT, nkt_total, DV], bf16, tag="v")
                nc.sync.dma_start(out=v_t, in_=v_d[h])
                nc.sync.dma_start(out=qt_t[:, c:], in_=qt_d[h, :, c:])
                nc.sync.dma_start(out=kt_t[:, c:], in_=kt_d[h, :, c:])
            else:
                nc.sync.dma_start(out=qt_t, in_=qt_d[h])
                nc.sync.dma_start(out=kt_t, in_=kt_d[h])
                v_t = v_pool.tile([KT, nkt_total, DV], bf16, tag="v")
                nc.sync.dma_start(out=v_t, in_=v_d[h])

            # Cross-block software pipeline (carried across heads too): a
            # block's trailing PV groups and its epilogue are emitted
            # interleaved between the next block's QK/exp groups, so the
            # in-order PE stream always has independent QK work between PV
            # ops that wait on exp results.
            lag_units = PV_LAG * (GT if TILE_DRAIN else 1)
            blocks = list(range(nqb - 1, -1, -1) if (BLOCK_DESC and h > 0)
                          else range(nqb))
            for j in blocks:
                # last block of the last head has no future QK work for the
                # PV lag to protect: drain eagerly to shrink the tail
                lag_eff = (1 if (h == hpc - 1 and j == blocks[-1])
                           else lag_units)
                o_ps = o_pool.tile([KT, 4 * DV], f32, tag="oacc",
                                   name=f"o_{rep}_{h}_{j}")
                nkt = 4 * (j + 1) if causal else nkt_total
                groups = [
                    list(range(g0, min(g0 + GT, nkt))) for g0 in range(0, nkt, GT)
                ]
                if causal and DIAG_REVERSE:
                    # higher-dg tile first inside diagonal groups: the e0 cut
                    # then skips more leading masked cols and the second
                    # tile's span is fully valid -> 256 fewer exp'd cols per
                    # block at zero instruction cost
                    groups = [list(reversed(ts)) if ts[0] - 4 * j >= 0 else ts
                              for ts in groups]
                # PV accumulation flags: start=True ONLY on the block's
                # very first PV matmul (start clears has_written for the
                # whole PSUM bank; later region first-touches overwrite),
                # stop=True on each region's last writer in emission order
                # (reversal changes who that is)
                ts0 = groups[0][0]
                dg0_blk = ts0 - 4 * j if causal else -1
                first_pv = (ts0, max(dg0_blk, 0))
                last_w = {}
                for ts in groups:
                    for t in ts:
                        dg = t - 4 * j if causal else -1
                        for s in range(4):
                            if dg > s:
                                continue
                            last_w[s] = t
                pts = []

                def _drain_one():
                    nonlocal carry_epi
                    if carry:
                        _emit_pv(nc, causal, carry.pop(0))
                        if not carry and carry_epi is not None:
                            _emit_epilogue(nc, out_pool, o_d, carry_epi, f32)
                            carry_epi = None
                    elif len(pts) >= lag_eff:
                        _emit_pv(nc, causal, pts.pop(0))

                for ts in groups:
                    w = len(ts) * QB
                    # skip the leading fully-masked columns of the group's
                    # first tile (nothing reads them); interior stale spans
                    # of later tiles still get exp'd harmlessly
                    dg0 = ts[0] - 4 * j if causal else -1
                    e0 = max(dg0, 0) * KT
                    st = st_pool.tile([KT, GT * QB], f32, tag="st")
                    masks = []
                    for i, t in enumerate(ts):
                        dg = t - 4 * j if causal else -1
                        # diagonal tiles: only queries >= dg*128 can see keys
                        # of tile t; skip the fully-masked cols (stale PSUM
                        # there is finite, exp'd harmlessly, never read by PV)
                        q0 = max(dg, 0) * KT
                        # PE row-group half for concurrency (DUAL_HALF off
                        # forces both tiles into half 0 -> serial: HW probe).
                        # Measured on HW: the overlap is real and worth ~390
                        # us/pass (848 serial vs 458 dual).
                        r0 = (i % 2) * D if DUAL_HALF else 0
                        nc.tensor.matmul(
                            st[:, i * QB + q0:(i + 1) * QB],
                            kt_t[r0:r0 + D, t * KT:(t + 1) * KT],
                            qt_t[r0:r0 + D, j * QB + q0:(j + 1) * QB],
                            start=True, stop=(dg < 0),
                        )
                        if dg >= 0:
                            if MASK_DEFER:
                                masks.append((i, dg))
                            else:
                                c0 = i * QB + dg * KT
                                nc.tensor.matmul(
                                    st[:, c0:c0 + KT],
                                    tri_t[:, :KT],
                                    tri_t[:, KT:2 * KT],
                                    start=False, stop=True,
                                )
                        if TILE_DRAIN:
                            # one PV tile after each QK tile: halves the PE
                            # burst quantum at the drain points
                            _drain_one()
                    for i, dg in masks:
                        # add -60 to masked (future-key) entries of the
                        # diagonal 128x128 block: identity.T @ mask_neg.
                        # Emitted AFTER both QK halves so the pair stays
                        # adjacent and the dual-half overlap window holds.
                        c0 = i * QB + dg * KT
                        nc.tensor.matmul(
                            st[:, c0:c0 + KT],
                            tri_t[:, :KT],
                            tri_t[:, KT:2 * KT],
                            start=False, stop=True,
                        )
                    if not TILE_DRAIN:
                        _drain_one()
                    pt = p_pool.tile([KT, GT * QB], bf16, tag="pt")
                    gi = groups.index(ts)
                    if DIAG_ENG is not None and causal and gi >= len(groups) - 2:
                        eng = DIAG_ENG[gi - (len(groups) - 2)]
                    else:
                        eng = EXP_PATTERN[gcounter % len(EXP_PATTERN)]
                        gcounter += 1
                    if eng == "dve":
                        nc.vector.tensor_scalar(
                            pt.bitcast(i16)[:, e0:w], st[:, e0:w], SCH_A, SCH_B,
                            MULT, ADD,
                        )
                    else:
                        nc.scalar.activation(pt[:, e0:w], st[:, e0:w], EXP)
                    if TILE_DRAIN:
                        for i, t in enumerate(ts):
                            pts.append((j, o_ps, [t], pt, v_t, i,
                                        first_pv, last_w))
                    else:
                        pts.append((j, o_ps, ts, pt, v_t, 0,
                                    first_pv, last_w))
                # previous block fully drained by now (it has fewer
                # groups than this block); stash this block's backlog
                for grp in carry:
                    _emit_pv(nc, causal, grp)
                if carry_epi is not None:
                    _emit_epilogue(nc, out_pool, o_d, carry_epi, f32)
                carry = pts
                carry_epi = (h, j, o_ps, rep)
        for grp in carry:
            _emit_pv(nc, causal, grp)
        if carry_epi is not None:
            _emit_epilogue(nc, out_pool, o_d, carry_epi, f32)


def _emit_pv(nc, causal, group):
    j, o_ps, ts, pt, v_t, i0, first_pv, last_w = group
    for i, t in enumerate(ts):
        ii = i0 + i  # column base of tile t within the group's pt tile
        dg = t - 4 * j if causal else -1
        for s in range(4):
            if dg > s:
                continue
            nc.tensor.matmul(
                o_ps[:, s * DV:(s + 1) * DV],
                pt[:, ii * QB + s * KT:ii * QB + (s + 1) * KT],
                v_t[:, t, :],
                start=((t, s) == first_pv),
                stop=(t == last_w[s]),
            )


def _emit_epilogue(nc, out_pool, o_d, epi, f32):
    h, j, o_ps, rep = epi
    recip = out_pool.tile([KT, 4], f32, tag="recip", name=f"r_{rep}_{h}_{j}")
    nc.vector.reciprocal(recip, o_ps[:, D::DV])
    out_t = out_pool.tile([KT, 4, D], f32, tag="out", name=f"t_{rep}_{h}_{j}")
    for s in range(4):
        nc.vector.tensor_scalar_mul(
            out_t[:, s], o_ps[:, s * DV:s * DV + D], recip[:, s:s + 1]
        )
    # issued from the idle GPSIMD queue so stores never block input-prefetch
    # DMAs queued on SP
    nc.gpsimd.dma_start(
        out=o_d[h, j * QB:(j + 1) * QB, :].rearrange("(s p) d -> p s d", s=4),
        in_=out_t,
    )


last_results = None  # BassKernelResults of the most recent run (for test.py)


def _make_in_maps(query, key, value):
    bf = ml_dtypes.bfloat16
    # cast to bf16 BEFORE the transposes: halves the bytes shuffled by the
    # non-contiguous copies below (host prep wall time)
    q4 = (np.asarray(query, dtype=np.float32) / math.sqrt(D)).astype(bf) \
        .reshape(B * H, S, D)
    k4 = np.asarray(key, dtype=np.float32).astype(bf).reshape(B * H, S, D)
    v4 = np.asarray(value, dtype=np.float32).astype(bf).reshape(B * H, S, D)
    # [identity | strict-lower-tri * -60]: operands of the mask-add matmul
    # (identity.T @ mask adds -60 where query < key inside a diagonal block)
    tri = np.concatenate(
        [np.eye(KT, dtype=np.float32),
         np.tril(np.full((KT, KT), -60.0, dtype=np.float32), -1)], axis=1
    ).astype(bf)

    in_maps = []
    for c in range(NCORES):
        sl = slice(c * HPC, (c + 1) * HPC)
        qt1 = np.ascontiguousarray(q4[sl].transpose(0, 2, 1))
        qt = np.concatenate([qt1, qt1], axis=1)
        kt1 = np.ascontiguousarray(k4[sl].transpose(0, 2, 1))
        kt = np.concatenate([kt1, kt1], axis=1)
        vb = v4[sl].reshape(HPC, NKT, KT, D)
        vones = np.empty((HPC, NKT, KT, DV), dtype=bf)
        vones[..., :D] = vb
        vones[..., D] = 1
        v_lay = np.ascontiguousarray(vones.transpose(0, 2, 1, 3))  # [HPC, KT, NKT, DV]
        in_maps.append({
            "qt": qt,
            "kt": kt,
            "v": v_lay,
            "tri": tri,
        })
    return in_maps


def _assemble(per_core_results):
    out = np.stack([r["o"] for r in per_core_results])  # [8, HPC, S, D]
    return np.ascontiguousarray(
        out.reshape(B, H, S, D)
    ).astype(np.float32)


def kernel(query, key, value, causal_mask):
    import os
    os.environ["BASS_NEVER_TRACE"] = "1"  # axon NTFF hook unavailable here
    from concourse.bass_utils import run_bass_kernel_spmd

    global last_results
    causal = bool(np.asarray(causal_mask).item())
    if causal not in _cache:
        _cache[causal] = _build(causal)
    nc = _cache[causal]

    in_maps = _make_in_maps(query, key, value)
    res = run_bass_kernel_spmd(nc, in_maps, core_ids=list(range(NCORES)))
    last_results = res
    return _assemble(res.results)

